# revision 1
# baseline (speedup 1.0000x reference)
"""Trainium2 Bass kernel: embedding gather + 2-layer MLP (relu), data-parallel on 8 cores.

Reference computation:
    x   = entity_embedding[idx0, idx1, :]        # [B, 128]  gather
    h   = relu(x @ w1.T + b1)                    # [B, 256]
    out = relu(h @ w2.T + b2)                    # [B, 86]

Shapes (hardcoded): entity_embedding [500000, 4, 128] f32, B = 131072.

Strategy:
  - Flatten the table to [2e6, 128]; flat row index = idx0*4 + idx1 (fits int32).
  - Shard the batch 8 ways (16384 rows/core); replicate table + weights.
  - Per core: gather rows via gpsimd indirect DMA into [128, j, 128] SBUF tiles
    (batch on partitions), transpose 128x128 sub-tiles on TensorE so features
    land on partitions, then run the MLP with batch on the free dim:
        hT[256h, b]  = w1 @ xT  (2 matmuls, N=512 free)
        outT[86, b]  = w2 @ hT  (2 accumulating matmuls)
    Biases are per-partition vectors in this orientation; relu+bias fuse into
    one ACT/DVE op per tile. Output is written transposed [86, 16384] per core
    and un-transposed on the host during unshard.
"""

import numpy as np
from contextlib import ExitStack

import concourse.bass as bass
import concourse.bacc as bacc
import concourse.tile as tile
from concourse import mybir
from concourse.bass_utils import run_bass_kernel_spmd
from concourse.masks import make_identity

F32 = mybir.dt.float32
I32 = mybir.dt.int32

N_CORES = 8
B = 131072
BC = B // N_CORES          # 16384 batch rows per core
FEAT = 128
NHID = 256
NOUT = 86
NROWS = 500000 * 4         # flattened table rows
P = 128
NJ = BC // P               # 128 j-columns of gathered rows per core
NJG = 16                   # j-columns per indirect-DMA gather call (2048 rows)
CHUNK_J = 4                # j-columns per MLP chunk (512 batch)
NCHUNK = NJ // CHUNK_J     # 32 chunks per core


def _build_program():
    nc = bacc.Bacc("TRN2", num_devices=N_CORES, num_swdge_queues=4)

    table = nc.dram_tensor("table", [NROWS, FEAT], F32, kind="ExternalInput").ap()
    idxs = nc.dram_tensor("idxs", [P, NJ], I32, kind="ExternalInput").ap()
    w1t = nc.dram_tensor("w1t", [FEAT, NHID], F32, kind="ExternalInput").ap()
    w2t = nc.dram_tensor("w2t", [NHID, NOUT], F32, kind="ExternalInput").ap()
    b1v = nc.dram_tensor("b1v", [NHID], F32, kind="ExternalInput").ap()
    b2v = nc.dram_tensor("b2v", [NOUT], F32, kind="ExternalInput").ap()
    outT = nc.dram_tensor("outT", [NOUT, BC], F32, kind="ExternalOutput").ap()

    with tile.TileContext(nc) as tc, ExitStack() as ctx:
        const = ctx.enter_context(tc.tile_pool(name="const", bufs=1))
        gpool = ctx.enter_context(tc.tile_pool(name="gather", bufs=4))
        xpool = ctx.enter_context(tc.tile_pool(name="xt", bufs=3))
        hpool = ctx.enter_context(tc.tile_pool(name="ht", bufs=3))
        opool = ctx.enter_context(tc.tile_pool(name="ot", bufs=3))
        psum = ctx.enter_context(tc.tile_pool(name="psum", bufs=2, space="PSUM"))

        idt = const.tile([P, P], F32)
        make_identity(nc, idt[:])

        w1t_t = const.tile([FEAT, NHID], F32)
        nc.sync.dma_start(w1t_t[:], w1t[:])
        w2t_t = const.tile([P, NHID // P, NOUT], F32)
        nc.sync.dma_start(w2t_t[:], w2t.rearrange("(k p) n -> p k n", p=P))
        b1_t = const.tile([P, NHID // P], F32)
        nc.sync.dma_start(b1_t[:], b1v.rearrange("(k p) -> p k", p=P))
        b2_t = const.tile([NOUT, 1], F32)
        nc.sync.dma_start(b2_t[:], b2v.rearrange("(n one) -> n one", one=1))
        idx_t = const.tile([P, NJ], I32)
        nc.sync.dma_start(idx_t[:], idxs[:])

        for c in range(NCHUNK):
            bcol = c * CHUNK_J * P  # column base in outT for this chunk
            # HW indirect DMA consumes ONE index per partition; gather the
            # chunk's 512 rows as CHUNK_J calls of 128 rows each.
            gt = gpool.tile([P, CHUNK_J, FEAT], F32)
            for i in range(CHUNK_J):
                j = c * CHUNK_J + i
                ginst = nc.gpsimd.indirect_dma_start(
                    out=gt[:, i, :],
                    out_offset=None,
                    in_=table[:],
                    in_offset=bass.IndirectOffsetOnAxis(
                        ap=idx_t[:, j:j + 1], axis=0
                    ),
                )
                # Spread descriptor generation over the 4 SWDGE queues
                # (parallel Q7 pairs + separate DMA rings).
                ginst.ins.queue = f"qPoolDynamic{i or ''}"
            if True:
                # Transpose 4x [128b, 128f] -> [128f, 128b] into one PSUM bank
                xtr = psum.tile([P, CHUNK_J * P], F32, tag="xtr")
                for i in range(CHUNK_J):
                    nc.tensor.transpose(
                        out=xtr[:, i * P:(i + 1) * P],
                        in_=gt[:, i, :],
                        identity=idt[:],
                    )
                xt = xpool.tile([P, CHUNK_J * P], F32)
                nc.vector.tensor_copy(out=xt[:], in_=xtr[:])

                # lin1: hT[k] = relu(w1[k] @ xT + b1[k]), k over 2 hid halves
                ht = hpool.tile([P, NHID // P, CHUNK_J * P], F32)
                for k in range(NHID // P):
                    hp = psum.tile([P, CHUNK_J * P], F32, tag=f"h{k}")
                    nc.tensor.matmul(
                        out=hp[:],
                        lhsT=w1t_t[:, k * P:(k + 1) * P],
                        rhs=xt[:],
                        start=True,
                        stop=True,
                    )
                    if k == 0:
                        nc.scalar.activation(
                            out=ht[:, k, :], in_=hp[:],
                            func=mybir.ActivationFunctionType.Relu,
                            bias=b1_t[:, k:k + 1],
                        )
                    else:
                        nc.vector.tensor_scalar(
                            out=ht[:, k, :], in0=hp[:],
                            scalar1=b1_t[:, k:k + 1], scalar2=0.0,
                            op0=mybir.AluOpType.add, op1=mybir.AluOpType.max,
                        )

                # lin2: outT = relu(w2 @ hT + b2), accumulate over 2 k-tiles
                op_ = psum.tile([NOUT, CHUNK_J * P], F32, tag="ot")
                for k in range(NHID // P):
                    nc.tensor.matmul(
                        out=op_[:],
                        lhsT=w2t_t[:, k, :],
                        rhs=ht[:, k, :],
                        start=(k == 0),
                        stop=(k == NHID // P - 1),
                    )
                ot = opool.tile([NOUT, CHUNK_J * P], F32)
                nc.scalar.activation(
                    out=ot[:], in_=op_[:],
                    func=mybir.ActivationFunctionType.Relu,
                    bias=b2_t[:],
                )
                nc.sync.dma_start(outT[:, bcol:bcol + CHUNK_J * P], ot[:])

    nc.compile()
    return nc


TRACE = False          # set by test harness to capture an NTFF profile
RUN_KWARGS = None      # extra kwargs for run_bass_kernel_spmd (test harness)
LAST = None            # last BassKernelResults (test harness reads exec_time_ns)

_SLOT_TO_BATCH = None


def _slot_map():
    """local batch index for gather slot (p, j): out column c*512 + i*128 + p
    where j = c*CHUNK_J + i must equal the local batch index."""
    global _SLOT_TO_BATCH
    if _SLOT_TO_BATCH is None:
        pp, jj = np.meshgrid(np.arange(P), np.arange(NJ), indexing="ij")
        cc = jj // CHUNK_J
        ii = jj % CHUNK_J
        _SLOT_TO_BATCH = cc * (CHUNK_J * P) + ii * P + pp  # [P, NJ]
    return _SLOT_TO_BATCH


def kernel(entity_embedding, w1, b1, w2, b2, idx0, idx1):
    table = np.ascontiguousarray(
        np.asarray(entity_embedding, dtype=np.float32).reshape(NROWS, FEAT)
    )
    flat_idx = (np.asarray(idx0, dtype=np.int64) * 4
                + np.asarray(idx1, dtype=np.int64)).astype(np.int32)
    w1t = np.ascontiguousarray(np.asarray(w1, dtype=np.float32).T)
    w2t = np.ascontiguousarray(np.asarray(w2, dtype=np.float32).T)
    b1v = np.ascontiguousarray(np.asarray(b1, dtype=np.float32))
    b2v = np.ascontiguousarray(np.asarray(b2, dtype=np.float32))

    slot = _slot_map()
    in_maps = []
    for core in range(N_CORES):
        local = flat_idx[core * BC:(core + 1) * BC]
        idxs = np.ascontiguousarray(local[slot])  # [P, NJ] int32
        in_maps.append({
            "table": table,
            "idxs": idxs,
            "w1t": w1t,
            "w2t": w2t,
            "b1v": b1v,
            "b2v": b2v,
        })

    nc = _build_program()
    global LAST
    res = run_bass_kernel_spmd(
        nc, in_maps, core_ids=list(range(N_CORES)), trace=TRACE,
        **(RUN_KWARGS or {}),
    )
    LAST = res
    out = np.empty((B, NOUT), dtype=np.float32)
    for core in range(N_CORES):
        out[core * BC:(core + 1) * BC] = res.results[core]["outT"].T
    return out


if __name__ == "__main__":
    rng = np.random.default_rng(0)
    ins = {
        "entity_embedding": rng.standard_normal((500000, 4, FEAT), dtype=np.float32),
        "w1": rng.standard_normal((NHID, FEAT), dtype=np.float32) / np.sqrt(FEAT),
        "b1": rng.standard_normal((NHID,), dtype=np.float32) / np.sqrt(FEAT),
        "w2": rng.standard_normal((NOUT, NHID), dtype=np.float32) / np.sqrt(NHID),
        "b2": rng.standard_normal((NOUT,), dtype=np.float32) / np.sqrt(NHID),
        "idx0": rng.integers(0, 500000, B).astype(np.int32),
        "idx1": rng.integers(0, 4, B).astype(np.int32),
    }
    out = kernel(**ins)
    x = ins["entity_embedding"].reshape(NROWS, FEAT)[
        ins["idx0"].astype(np.int64) * 4 + ins["idx1"]]
    h = np.maximum(x @ ins["w1"].T + ins["b1"], 0.0)
    ref = np.maximum(h @ ins["w2"].T + ins["b2"], 0.0)
    err = np.abs(out - ref).max() / max(np.abs(ref).max(), 1e-9)
    print("rel err:", err)



# revision 3
# speedup vs baseline: 1.9445x; 1.9445x over previous
"""Trainium2 Bass kernel: embedding gather + 2-layer MLP (relu), 8 cores.

Reference computation:
    x   = entity_embedding[idx0, idx1, :]        # [B, 128]  gather
    h   = relu(x @ w1.T + b1)                    # [B, 256]
    out = relu(h @ w2.T + b2)                    # [B, 86]

Shapes (hardcoded): entity_embedding [500000, 4, 128] f32, B = 131072.

Strategy (v2):
  - Cast the table to bf16 on the host (tolerance is 2e-2; bf16 end-to-end
    error is ~5e-3).  Halves gather bytes and runs the MLP at bf16 matmul
    rate.
  - Sort the flattened indices on the host; core c takes sorted positions
    [c*16384, (c+1)*16384) — exactly 16384 rows/core, and each core's rows
    span a narrow contiguous band of the table.
  - Gather with gpsimd.dma_gather(transpose=True): one call gathers up to
    1920 rows with int16 in-window indices and lands them FEATURE-MAJOR
    (features on partitions), so no TensorE transposes and no PSUM->SBUF
    copies are needed.  9 calls/core spread over the 4 SWDGE queues replace
    the baseline's 128 serialized indirect DMAs (994 ns fixed cost each).
    The int16 window limit (32768 rows) is handled by building a per-core
    DRAM table copy out of 9 host-chosen 32768-row windows, so the program
    itself is fully static and identical on every core.
  - MLP with batch on the free dim, 512-column chunks:
        hT[256h, n] = relu(w1 @ xT + b1)   2 matmuls -> 2 PSUM banks
        oT[86, n]   = relu(w2 @ hT + b2)   2 accumulating matmuls
    Bias+relu fuse into one ACT/DVE op per tile (engines alternated to
    balance load).  Output is written bf16 [86, 16384] per core; the host
    casts to f32 and un-permutes the sort.
"""

import numpy as np
from contextlib import ExitStack

import ml_dtypes

import concourse.bass as bass
import concourse.bacc as bacc
import concourse.tile as tile
from concourse import mybir
from concourse.bass_utils import run_bass_kernel_spmd

F32 = mybir.dt.float32
BF16 = mybir.dt.bfloat16
I16 = mybir.dt.int16
BF16_NP = ml_dtypes.bfloat16

N_CORES = 8
B = 131072
BC = B // N_CORES          # 16384 batch rows per core
FEAT = 128
NHID = 256
NOUT = 86
NROWS = 500000 * 4         # flattened table rows
P = 128
WIN = 32768                # int16 index window per dma_gather call

# Static call plan: sizes must be multiples of 128 (dma_gather transpose) and
# at most ~990 (the SWDGE descriptor ring holds ~1024 descriptors per
# instruction: N + 2*16 overhead must fit).  896 sorted uniform indices span
# ~13.7k rows, far below the 32768-row int16 window.
CALL_SIZES = [896] * 18 + [256]
assert sum(CALL_SIZES) == BC
CALL_OFFS = np.concatenate([[0], np.cumsum(CALL_SIZES)]).astype(int)
NCALLS = len(CALL_SIZES)
# Fallback plan if some window overflows (astronomically unlikely).
CALL_SIZES_SAFE = [512] * 32

CHUNK = 512                # MLP chunk width (one PSUM bank of f32)
NCHUNK = BC // CHUNK


def _build_program(call_sizes):
    call_offs = np.concatenate([[0], np.cumsum(call_sizes)]).astype(int)
    ncalls = len(call_sizes)
    nidxcol = BC // 16

    nc = bacc.Bacc("TRN2", num_devices=N_CORES, num_swdge_queues=4)

    ltab = nc.dram_tensor("ltab", [ncalls * WIN, FEAT], BF16,
                          kind="ExternalInput").ap()
    idxs = nc.dram_tensor("idxs", [P, nidxcol], I16, kind="ExternalInput").ap()
    w1t = nc.dram_tensor("w1t", [FEAT, NHID], BF16, kind="ExternalInput").ap()
    w2t = nc.dram_tensor("w2t", [P, NHID // P, NOUT], BF16,
                         kind="ExternalInput").ap()
    b1v = nc.dram_tensor("b1v", [P, NHID // P], F32, kind="ExternalInput").ap()
    b2v = nc.dram_tensor("b2v", [NOUT, 1], F32, kind="ExternalInput").ap()
    outT = nc.dram_tensor("outT", [NOUT, BC], BF16, kind="ExternalOutput").ap()

    with tile.TileContext(nc) as tc, ExitStack() as ctx:
        const = ctx.enter_context(tc.tile_pool(name="const", bufs=1))
        xpool = ctx.enter_context(tc.tile_pool(name="xt", bufs=1))
        hpool = ctx.enter_context(tc.tile_pool(name="ht", bufs=3))
        opool = ctx.enter_context(tc.tile_pool(name="ot", bufs=3))
        psum = ctx.enter_context(tc.tile_pool(name="psum", bufs=2, space="PSUM"))

        w1t_t = const.tile([FEAT, NHID], BF16)
        nc.sync.dma_start(w1t_t[:], w1t[:])
        w2t_t = const.tile([P, NHID // P, NOUT], BF16)
        nc.sync.dma_start(w2t_t[:], w2t[:])
        b1_t = const.tile([P, NHID // P], F32)
        nc.sync.dma_start(b1_t[:], b1v[:])
        b2_t = const.tile([NOUT, 1], F32)
        nc.sync.dma_start(b2_t[:], b2v[:])
        idx_t = const.tile([P, nidxcol], I16)
        nc.sync.dma_start(idx_t[:], idxs[:])

        # Whole-core gathered activations, feature-major: xt[f, 0, n].
        xt = xpool.tile([P, 1, BC], BF16)

        for k in range(ncalls):
            s, n = int(call_offs[k]), int(call_sizes[k])
            nc.gpsimd.dma_gather(
                out_ap=xt[:, :, s:s + n],
                in_ap=ltab[k * WIN:(k + 1) * WIN, :],
                idxs_ap=idx_t[:, s // 16:(s + n) // 16],
                num_idxs=n,
                num_idxs_reg=n,
                elem_size=FEAT,
                transpose=True,
                queue_num=k % 4,
            )

        for c in range(NCHUNK):
            col = c * CHUNK
            rhs = xt[:, 0, col:col + CHUNK]

            ht = hpool.tile([P, NHID // P, CHUNK], BF16)
            for k in range(NHID // P):
                hp = psum.tile([P, CHUNK], F32, tag=f"h{k}")
                nc.tensor.matmul(
                    out=hp[:],
                    lhsT=w1t_t[:, k * P:(k + 1) * P],
                    rhs=rhs,
                    start=True,
                    stop=True,
                )
                if k == 0:
                    nc.scalar.activation(
                        out=ht[:, k, :], in_=hp[:],
                        func=mybir.ActivationFunctionType.Relu,
                        bias=b1_t[:, k:k + 1],
                    )
                else:
                    nc.vector.tensor_scalar(
                        out=ht[:, k, :], in0=hp[:],
                        scalar1=b1_t[:, k:k + 1], scalar2=0.0,
                        op0=mybir.AluOpType.add, op1=mybir.AluOpType.max,
                    )

            op_ = psum.tile([NOUT, CHUNK], F32, tag="ot")
            for k in range(NHID // P):
                nc.tensor.matmul(
                    out=op_[:],
                    lhsT=w2t_t[:, k, :],
                    rhs=ht[:, k, :],
                    start=(k == 0),
                    stop=(k == NHID // P - 1),
                )
            ot = opool.tile([NOUT, CHUNK], BF16)
            if c % 2 == 0:
                nc.vector.tensor_scalar(
                    out=ot[:], in0=op_[:],
                    scalar1=b2_t[:], scalar2=0.0,
                    op0=mybir.AluOpType.add, op1=mybir.AluOpType.max,
                )
            else:
                nc.scalar.activation(
                    out=ot[:], in_=op_[:],
                    func=mybir.ActivationFunctionType.Relu,
                    bias=b2_t[:],
                )
            nc.sync.dma_start(outT[:, col:col + CHUNK], ot[:])

    nc.compile()
    return nc


TRACE = False          # set by test harness to capture an NTFF profile
RUN_KWARGS = None      # extra kwargs for run_bass_kernel_spmd (test harness)
LAST = None            # last BassKernelResults (test harness reads exec_time_ns)


def _plan_windows(sidx, call_sizes):
    """Per-core window bases for each gather call; None if a window overflows."""
    call_offs = np.concatenate([[0], np.cumsum(call_sizes)]).astype(int)
    bases = np.empty((N_CORES, len(call_sizes)), dtype=np.int64)
    for c in range(N_CORES):
        seg = sidx[c * BC:(c + 1) * BC]
        for k, n in enumerate(call_sizes):
            s = int(call_offs[k])
            lo, hi = int(seg[s]), int(seg[s + n - 1])
            if hi - lo >= WIN:
                return None
            bases[c, k] = min(lo, NROWS - WIN)
    return bases


def kernel(entity_embedding, w1, b1, w2, b2, idx0, idx1):
    table = np.asarray(entity_embedding, dtype=np.float32).reshape(NROWS, FEAT)
    table_bf = table.astype(BF16_NP)
    flat_idx = (np.asarray(idx0, dtype=np.int64) * 4
                + np.asarray(idx1, dtype=np.int64))

    order = np.argsort(flat_idx, kind="stable")
    sidx = flat_idx[order]

    call_sizes = CALL_SIZES
    bases = _plan_windows(sidx, call_sizes)
    if bases is None:
        call_sizes = CALL_SIZES_SAFE
        bases = _plan_windows(sidx, call_sizes)
        assert bases is not None, "index windows overflow even at 1024/call"
    call_offs = np.concatenate([[0], np.cumsum(call_sizes)]).astype(int)

    w1t = np.ascontiguousarray(np.asarray(w1, dtype=np.float32).T).astype(BF16_NP)
    w2t = np.ascontiguousarray(
        np.asarray(w2, dtype=np.float32).T.reshape(NHID // P, P, NOUT)
        .transpose(1, 0, 2)).astype(BF16_NP)
    b1v = np.ascontiguousarray(
        np.asarray(b1, dtype=np.float32).reshape(NHID // P, P).T)
    b2v = np.ascontiguousarray(np.asarray(b2, dtype=np.float32).reshape(NOUT, 1))

    nidxcol = BC // 16
    in_maps = []
    for c in range(N_CORES):
        seg = sidx[c * BC:(c + 1) * BC]
        ltab = np.concatenate(
            [table_bf[bases[c, k]:bases[c, k] + WIN] for k in range(len(call_sizes))],
            axis=0)
        idx16 = np.empty((16, nidxcol), dtype=np.int16)
        for k, n in enumerate(call_sizes):
            s = int(call_offs[k])
            local = (seg[s:s + n] - bases[c, k]).astype(np.int16)
            idx16[:, s // 16:(s + n) // 16] = local.reshape(n // 16, 16).T
        idxs = np.ascontiguousarray(np.tile(idx16, (P // 16, 1)))
        in_maps.append({
            "ltab": ltab,
            "idxs": idxs,
            "w1t": w1t,
            "w2t": w2t,
            "b1v": b1v,
            "b2v": b2v,
        })

    nc = _build_program(call_sizes)
    global LAST
    res = run_bass_kernel_spmd(
        nc, in_maps, core_ids=list(range(N_CORES)), trace=TRACE,
        **(RUN_KWARGS or {}),
    )
    LAST = res
    sorted_out = np.empty((B, NOUT), dtype=np.float32)
    for c in range(N_CORES):
        sorted_out[c * BC:(c + 1) * BC] = (
            np.asarray(res.results[c]["outT"]).astype(np.float32).T)
    out = np.empty((B, NOUT), dtype=np.float32)
    out[order] = sorted_out
    return out


if __name__ == "__main__":
    rng = np.random.default_rng(0)
    ins = {
        "entity_embedding": rng.standard_normal((500000, 4, FEAT), dtype=np.float32),
        "w1": rng.standard_normal((NHID, FEAT), dtype=np.float32) / np.sqrt(FEAT),
        "b1": rng.standard_normal((NHID,), dtype=np.float32) / np.sqrt(FEAT),
        "w2": rng.standard_normal((NOUT, NHID), dtype=np.float32) / np.sqrt(NHID),
        "b2": rng.standard_normal((NOUT,), dtype=np.float32) / np.sqrt(NHID),
        "idx0": rng.integers(0, 500000, B).astype(np.int32),
        "idx1": rng.integers(0, 4, B).astype(np.int32),
    }
    out = kernel(**ins)
    x = ins["entity_embedding"].reshape(NROWS, FEAT)[
        ins["idx0"].astype(np.int64) * 4 + ins["idx1"]]
    h = np.maximum(x @ ins["w1"].T + ins["b1"], 0.0)
    ref = np.maximum(h @ ins["w2"].T + ins["b2"], 0.0)
    err = np.abs(out - ref).max() / max(np.abs(ref).max(), 1e-9)
    print("rel err:", err)


# revision 4
# speedup vs baseline: 1.9464x; 1.0010x over previous
"""Trainium2 Bass kernel: embedding gather + 2-layer MLP (relu), 8 cores.

Reference computation:
    x   = entity_embedding[idx0, idx1, :]        # [B, 128]  gather
    h   = relu(x @ w1.T + b1)                    # [B, 256]
    out = relu(h @ w2.T + b2)                    # [B, 86]

Shapes (hardcoded): entity_embedding [500000, 4, 128] f32, B = 131072.

Strategy (v2):
  - Cast the table to bf16 on the host (tolerance is 2e-2; bf16 end-to-end
    error is ~5e-3).  Halves gather bytes and runs the MLP at bf16 matmul
    rate.
  - Sort the flattened indices on the host; core c takes sorted positions
    [c*16384, (c+1)*16384) — exactly 16384 rows/core, and each core's rows
    span a narrow contiguous band of the table.
  - Gather with gpsimd.dma_gather(transpose=True): one call gathers up to
    1920 rows with int16 in-window indices and lands them FEATURE-MAJOR
    (features on partitions), so no TensorE transposes and no PSUM->SBUF
    copies are needed.  9 calls/core spread over the 4 SWDGE queues replace
    the baseline's 128 serialized indirect DMAs (994 ns fixed cost each).
    The int16 window limit (32768 rows) is handled by building a per-core
    DRAM table copy out of 9 host-chosen 32768-row windows, so the program
    itself is fully static and identical on every core.
  - MLP with batch on the free dim, 512-column chunks:
        hT[256h, n] = relu(w1 @ xT + b1)   2 matmuls -> 2 PSUM banks
        oT[86, n]   = relu(w2 @ hT + b2)   2 accumulating matmuls
    Bias+relu fuse into one ACT/DVE op per tile (engines alternated to
    balance load).  Output is written bf16 [86, 16384] per core; the host
    casts to f32 and un-permutes the sort.
"""

import numpy as np
from contextlib import ExitStack

import ml_dtypes

import concourse.bass as bass
import concourse.bacc as bacc
import concourse.tile as tile
from concourse import mybir
from concourse.bass_utils import run_bass_kernel_spmd

F32 = mybir.dt.float32
BF16 = mybir.dt.bfloat16
I16 = mybir.dt.int16
BF16_NP = ml_dtypes.bfloat16

N_CORES = 8
B = 131072
BC = B // N_CORES          # 16384 batch rows per core
FEAT = 128
NHID = 256
NOUT = 86
NROWS = 500000 * 4         # flattened table rows
P = 128
WIN = 32768                # int16 index window per dma_gather call

# Static call plan: sizes must be multiples of 128 (dma_gather transpose) and
# at most ~990 (the SWDGE descriptor ring holds ~1024 descriptors per
# instruction: N + 2*16 overhead must fit).  896 sorted uniform indices span
# ~13.7k rows, far below the 32768-row int16 window.
CALL_SIZES = [896] * 18 + [256]
assert sum(CALL_SIZES) == BC
CALL_OFFS = np.concatenate([[0], np.cumsum(CALL_SIZES)]).astype(int)
NCALLS = len(CALL_SIZES)
# Fallback plan if some window overflows (astronomically unlikely).
CALL_SIZES_SAFE = [512] * 32

CHUNK = 512                # MLP chunk width (one PSUM bank of f32)
NCHUNK = BC // CHUNK


def _build_program(call_sizes):
    call_offs = np.concatenate([[0], np.cumsum(call_sizes)]).astype(int)
    ncalls = len(call_sizes)
    nidxcol = BC // 16

    nc = bacc.Bacc("TRN2", num_devices=N_CORES, num_swdge_queues=4)

    ltab = nc.dram_tensor("ltab", [ncalls * WIN, FEAT], BF16,
                          kind="ExternalInput").ap()
    idxs = nc.dram_tensor("idxs", [P, nidxcol], I16, kind="ExternalInput").ap()
    w1t = nc.dram_tensor("w1t", [FEAT, NHID], BF16, kind="ExternalInput").ap()
    w2t = nc.dram_tensor("w2t", [P, NHID // P, NOUT], BF16,
                         kind="ExternalInput").ap()
    b1v = nc.dram_tensor("b1v", [P, NHID // P], F32, kind="ExternalInput").ap()
    b2v = nc.dram_tensor("b2v", [NOUT, 1], F32, kind="ExternalInput").ap()
    outT = nc.dram_tensor("outT", [NOUT, BC], BF16, kind="ExternalOutput").ap()

    with tile.TileContext(nc) as tc, ExitStack() as ctx:
        const = ctx.enter_context(tc.tile_pool(name="const", bufs=1))
        xpool = ctx.enter_context(tc.tile_pool(name="xt", bufs=1))
        hpool = ctx.enter_context(tc.tile_pool(name="ht", bufs=3))
        opool = ctx.enter_context(tc.tile_pool(name="ot", bufs=4))
        hpsum = ctx.enter_context(tc.tile_pool(name="hpsum", bufs=2, space="PSUM"))
        opsum = ctx.enter_context(tc.tile_pool(name="opsum", bufs=3, space="PSUM"))

        # idx tile loaded in 4 column-slices so early gather calls can start
        # before the whole 256 KB index load lands.
        idx_t = const.tile([P, nidxcol], I16)
        nsl = 4
        slc = nidxcol // nsl
        for i in range(nsl):
            nc.scalar.dma_start(idx_t[:, i * slc:(i + 1) * slc],
                                idxs[:, i * slc:(i + 1) * slc])
        w1t_t = const.tile([FEAT, NHID], BF16)
        nc.sync.dma_start(w1t_t[:], w1t[:])
        w2t_t = const.tile([P, NHID // P, NOUT], BF16)
        nc.sync.dma_start(w2t_t[:], w2t[:])
        b1_t = const.tile([P, NHID // P], F32)
        nc.sync.dma_start(b1_t[:], b1v[:])
        b2_t = const.tile([NOUT, 1], F32)
        nc.sync.dma_start(b2_t[:], b2v[:])

        # Whole-core gathered activations, feature-major: xt[f, 0, n].
        xt = xpool.tile([P, 1, BC], BF16)

        for k in range(ncalls):
            s, n = int(call_offs[k]), int(call_sizes[k])
            nc.gpsimd.dma_gather(
                out_ap=xt[:, :, s:s + n],
                in_ap=ltab[k * WIN:(k + 1) * WIN, :],
                idxs_ap=idx_t[:, s // 16:(s + n) // 16],
                num_idxs=n,
                num_idxs_reg=n,
                elem_size=FEAT,
                transpose=True,
                queue_num=k % 4,
            )

        # Software-pipelined MLP: lin2 for chunk c-1 issues after lin1 for
        # chunk c, so the PE never stalls waiting on the ACT/DVE relu of the
        # chunk it just produced.
        hts = [None] * NCHUNK

        def lin1(c):
            col = c * CHUNK
            ht = hpool.tile([P, NHID // P, CHUNK], BF16)
            hts[c] = ht
            for k in range(NHID // P):
                hp = hpsum.tile([P, CHUNK], F32, tag=f"h{k}")
                nc.tensor.matmul(
                    out=hp[:],
                    lhsT=w1t_t[:, k * P:(k + 1) * P],
                    rhs=xt[:, 0, col:col + CHUNK],
                    start=True,
                    stop=True,
                )
                if k == 0:
                    nc.scalar.activation(
                        out=ht[:, k, :], in_=hp[:],
                        func=mybir.ActivationFunctionType.Relu,
                        bias=b1_t[:, k:k + 1],
                    )
                else:
                    nc.vector.tensor_scalar(
                        out=ht[:, k, :], in0=hp[:],
                        scalar1=b1_t[:, k:k + 1], scalar2=0.0,
                        op0=mybir.AluOpType.add, op1=mybir.AluOpType.max,
                    )

        def lin2(c):
            col = c * CHUNK
            ht = hts[c]
            op_ = opsum.tile([NOUT, CHUNK], F32, tag="ot")
            for k in range(NHID // P):
                nc.tensor.matmul(
                    out=op_[:],
                    lhsT=w2t_t[:, k, :],
                    rhs=ht[:, k, :],
                    start=(k == 0),
                    stop=(k == NHID // P - 1),
                )
            ot = opool.tile([NOUT, CHUNK], BF16)
            if c % 2 == 0:
                nc.vector.tensor_scalar(
                    out=ot[:], in0=op_[:],
                    scalar1=b2_t[:], scalar2=0.0,
                    op0=mybir.AluOpType.add, op1=mybir.AluOpType.max,
                )
            else:
                nc.scalar.activation(
                    out=ot[:], in_=op_[:],
                    func=mybir.ActivationFunctionType.Relu,
                    bias=b2_t[:],
                )
            # Alternate the two HWDGE rings so output writes run ~2x faster.
            eng = nc.sync if c % 2 == 0 else nc.scalar
            eng.dma_start(outT[:, col:col + CHUNK], ot[:])

        for c in range(NCHUNK):
            lin1(c)
            if c >= 1:
                lin2(c - 1)
        lin2(NCHUNK - 1)

    nc.compile()
    return nc


TRACE = False          # set by test harness to capture an NTFF profile
RUN_KWARGS = None      # extra kwargs for run_bass_kernel_spmd (test harness)
LAST = None            # last BassKernelResults (test harness reads exec_time_ns)


def _plan_windows(sidx, call_sizes):
    """Per-core window bases for each gather call; None if a window overflows."""
    call_offs = np.concatenate([[0], np.cumsum(call_sizes)]).astype(int)
    bases = np.empty((N_CORES, len(call_sizes)), dtype=np.int64)
    for c in range(N_CORES):
        seg = sidx[c * BC:(c + 1) * BC]
        for k, n in enumerate(call_sizes):
            s = int(call_offs[k])
            lo, hi = int(seg[s]), int(seg[s + n - 1])
            if hi - lo >= WIN:
                return None
            bases[c, k] = min(lo, NROWS - WIN)
    return bases


def kernel(entity_embedding, w1, b1, w2, b2, idx0, idx1):
    table = np.asarray(entity_embedding, dtype=np.float32).reshape(NROWS, FEAT)
    table_bf = table.astype(BF16_NP)
    flat_idx = (np.asarray(idx0, dtype=np.int64) * 4
                + np.asarray(idx1, dtype=np.int64))

    order = np.argsort(flat_idx, kind="stable")
    sidx = flat_idx[order]

    call_sizes = CALL_SIZES
    bases = _plan_windows(sidx, call_sizes)
    if bases is None:
        call_sizes = CALL_SIZES_SAFE
        bases = _plan_windows(sidx, call_sizes)
        assert bases is not None, "index windows overflow even at 1024/call"
    call_offs = np.concatenate([[0], np.cumsum(call_sizes)]).astype(int)

    w1t = np.ascontiguousarray(np.asarray(w1, dtype=np.float32).T).astype(BF16_NP)
    w2t = np.ascontiguousarray(
        np.asarray(w2, dtype=np.float32).T.reshape(NHID // P, P, NOUT)
        .transpose(1, 0, 2)).astype(BF16_NP)
    b1v = np.ascontiguousarray(
        np.asarray(b1, dtype=np.float32).reshape(NHID // P, P).T)
    b2v = np.ascontiguousarray(np.asarray(b2, dtype=np.float32).reshape(NOUT, 1))

    nidxcol = BC // 16
    in_maps = []
    for c in range(N_CORES):
        seg = sidx[c * BC:(c + 1) * BC]
        ltab = np.concatenate(
            [table_bf[bases[c, k]:bases[c, k] + WIN] for k in range(len(call_sizes))],
            axis=0)
        idx16 = np.empty((16, nidxcol), dtype=np.int16)
        for k, n in enumerate(call_sizes):
            s = int(call_offs[k])
            local = (seg[s:s + n] - bases[c, k]).astype(np.int16)
            idx16[:, s // 16:(s + n) // 16] = local.reshape(n // 16, 16).T
        idxs = np.ascontiguousarray(np.tile(idx16, (P // 16, 1)))
        in_maps.append({
            "ltab": ltab,
            "idxs": idxs,
            "w1t": w1t,
            "w2t": w2t,
            "b1v": b1v,
            "b2v": b2v,
        })

    nc = _build_program(call_sizes)
    global LAST
    res = run_bass_kernel_spmd(
        nc, in_maps, core_ids=list(range(N_CORES)), trace=TRACE,
        **(RUN_KWARGS or {}),
    )
    LAST = res
    sorted_out = np.empty((B, NOUT), dtype=np.float32)
    for c in range(N_CORES):
        sorted_out[c * BC:(c + 1) * BC] = (
            np.asarray(res.results[c]["outT"]).astype(np.float32).T)
    out = np.empty((B, NOUT), dtype=np.float32)
    out[order] = sorted_out
    return out


if __name__ == "__main__":
    rng = np.random.default_rng(0)
    ins = {
        "entity_embedding": rng.standard_normal((500000, 4, FEAT), dtype=np.float32),
        "w1": rng.standard_normal((NHID, FEAT), dtype=np.float32) / np.sqrt(FEAT),
        "b1": rng.standard_normal((NHID,), dtype=np.float32) / np.sqrt(FEAT),
        "w2": rng.standard_normal((NOUT, NHID), dtype=np.float32) / np.sqrt(NHID),
        "b2": rng.standard_normal((NOUT,), dtype=np.float32) / np.sqrt(NHID),
        "idx0": rng.integers(0, 500000, B).astype(np.int32),
        "idx1": rng.integers(0, 4, B).astype(np.int32),
    }
    out = kernel(**ins)
    x = ins["entity_embedding"].reshape(NROWS, FEAT)[
        ins["idx0"].astype(np.int64) * 4 + ins["idx1"]]
    h = np.maximum(x @ ins["w1"].T + ins["b1"], 0.0)
    ref = np.maximum(h @ ins["w2"].T + ins["b2"], 0.0)
    err = np.abs(out - ref).max() / max(np.abs(ref).max(), 1e-9)
    print("rel err:", err)


# revision 10
# speedup vs baseline: 1.9738x; 1.0141x over previous
"""Trainium2 Bass kernel: embedding gather + 2-layer MLP (relu), 8 cores.

Reference computation:
    x   = entity_embedding[idx0, idx1, :]        # [B, 128]  gather
    h   = relu(x @ w1.T + b1)                    # [B, 256]
    out = relu(h @ w2.T + b2)                    # [B, 86]

Shapes (hardcoded): entity_embedding [500000, 4, 128] f32, B = 131072.

Strategy (v2):
  - Cast the table to bf16 on the host (tolerance is 2e-2; bf16 end-to-end
    error is ~5e-3).  Halves gather bytes and runs the MLP at bf16 matmul
    rate.
  - Sort the flattened indices on the host; core c takes sorted positions
    [c*16384, (c+1)*16384) — exactly 16384 rows/core, and each core's rows
    span a narrow contiguous band of the table.
  - Gather with gpsimd.dma_gather(transpose=True): one call gathers up to
    1920 rows with int16 in-window indices and lands them FEATURE-MAJOR
    (features on partitions), so no TensorE transposes and no PSUM->SBUF
    copies are needed.  9 calls/core spread over the 4 SWDGE queues replace
    the baseline's 128 serialized indirect DMAs (994 ns fixed cost each).
    The int16 window limit (32768 rows) is handled by building a per-core
    DRAM table copy out of 9 host-chosen 32768-row windows, so the program
    itself is fully static and identical on every core.
  - MLP with batch on the free dim, 512-column chunks:
        hT[256h, n] = relu(w1 @ xT + b1)   2 matmuls -> 2 PSUM banks
        oT[86, n]   = relu(w2 @ hT + b2)   2 accumulating matmuls
    Bias+relu fuse into one ACT/DVE op per tile (engines alternated to
    balance load).  Output is written bf16 [86, 16384] per core; the host
    casts to f32 and un-permutes the sort.
"""

import numpy as np
from contextlib import ExitStack

import ml_dtypes

import concourse.bass as bass
import concourse.bacc as bacc
import concourse.tile as tile
from concourse import mybir
from concourse.bass_utils import run_bass_kernel_spmd

F32 = mybir.dt.float32
BF16 = mybir.dt.bfloat16
I16 = mybir.dt.int16
BF16_NP = ml_dtypes.bfloat16

N_CORES = 8
B = 131072
BC = B // N_CORES          # 16384 batch rows per core
FEAT = 128
NHID = 256
NOUT = 86
NROWS = 500000 * 4         # flattened table rows
P = 128
WIN = 32768                # int16 index window per dma_gather call

# Static call plan: sizes must be multiples of 128 (dma_gather transpose) and
# at most ~990 (the SWDGE descriptor ring holds ~1024 descriptors per
# instruction: N + 2*16 overhead must fit).  896 sorted uniform indices span
# ~13.7k rows, far below the 32768-row int16 window.
CALL_SIZES = [896] * 18 + [256]
assert sum(CALL_SIZES) == BC
CALL_OFFS = np.concatenate([[0], np.cumsum(CALL_SIZES)]).astype(int)
NCALLS = len(CALL_SIZES)
# Fallback plan if some window overflows (astronomically unlikely).
CALL_SIZES_SAFE = [512] * 32

CHUNK = 512                # MLP chunk width (one PSUM bank of f32)
NCHUNK = BC // CHUNK


def _build_program(call_sizes):
    call_offs = np.concatenate([[0], np.cumsum(call_sizes)]).astype(int)
    ncalls = len(call_sizes)
    nidxcol = BC // 16

    nc = bacc.Bacc("TRN2", num_devices=N_CORES, num_swdge_queues=4)

    ltab = nc.dram_tensor("ltab", [ncalls * WIN, FEAT], BF16,
                          kind="ExternalInput").ap()
    idxs = nc.dram_tensor("idxs", [P, nidxcol], I16, kind="ExternalInput").ap()
    w1t = nc.dram_tensor("w1t", [FEAT, NHID], BF16, kind="ExternalInput").ap()
    w2t = nc.dram_tensor("w2t", [P, NHID // P, NOUT], BF16,
                         kind="ExternalInput").ap()
    b1v = nc.dram_tensor("b1v", [P, NHID // P], F32, kind="ExternalInput").ap()
    b2v = nc.dram_tensor("b2v", [NOUT, 1], F32, kind="ExternalInput").ap()
    outT = nc.dram_tensor("outT", [NOUT, BC], BF16, kind="ExternalOutput").ap()

    with tile.TileContext(nc) as tc, ExitStack() as ctx:
        const = ctx.enter_context(tc.tile_pool(name="const", bufs=1))
        xpool = ctx.enter_context(tc.tile_pool(name="xt", bufs=1))
        hpool = ctx.enter_context(tc.tile_pool(name="ht", bufs=3))
        opool = ctx.enter_context(tc.tile_pool(name="ot", bufs=4))
        hpsum = ctx.enter_context(tc.tile_pool(name="hpsum", bufs=1, space="PSUM"))
        opsum = ctx.enter_context(tc.tile_pool(name="opsum", bufs=2, space="PSUM"))

        # The dma_gather ucode for queue q reads indices from the 32-partition
        # group [32q, 32q+32) (one 16-row band per Q7 cpu of the pair);
        # CoreSim reads partitions 0-15.  The DRAM tensor holds 8 identical
        # 16-row replicas, so every group sees the same data.  Split the load
        # into four 32-line strips across both HWDGE rings to shorten the
        # startup serialization.
        idx_t = const.tile([P, nidxcol], I16)
        for i, pbase in enumerate((0, 32, 64, 96)):
            eng = nc.sync if i % 2 == 0 else nc.scalar
            eng.dma_start(idx_t[pbase:pbase + 32, :], idxs[pbase:pbase + 32, :])
        w1t_t = const.tile([FEAT, NHID], BF16)
        nc.sync.dma_start(w1t_t[:], w1t[:])
        w2t_t = const.tile([P, NHID // P, NOUT], BF16)
        nc.sync.dma_start(w2t_t[:], w2t[:])
        b1_t = const.tile([P, NHID // P], F32)
        nc.sync.dma_start(b1_t[:], b1v[:])
        b2_t = const.tile([NOUT, 1], F32)
        nc.sync.dma_start(b2_t[:], b2v[:])

        # Whole-core gathered activations, feature-major: xt[f, 0, n].
        xt = xpool.tile([P, 1, BC], BF16)

        for k in range(ncalls):
            s, n = int(call_offs[k]), int(call_sizes[k])
            nc.gpsimd.dma_gather(
                out_ap=xt[:, :, s:s + n],
                in_ap=ltab[k * WIN:(k + 1) * WIN, :],
                idxs_ap=idx_t[:, s // 16:(s + n) // 16],
                num_idxs=n,
                num_idxs_reg=n,
                elem_size=FEAT,
                transpose=True,
                queue_num=k % 4,
            )

        # Software-pipelined MLP over PAIRS of 512-column chunks (1024 cols per
        # pair): lin2 for pair p-1 issues after lin1 for pair p, so the PE
        # never stalls on the ACT/DVE relu of the chunk it just produced.
        # Within a pair, consecutive matmuls share the same stationary weights
        # (amortizes LDWEIGHTS if the compiler dedups the reload).
        PAIR = 2 * CHUNK
        NPAIR = BC // PAIR
        hts = [None] * NPAIR

        def lin1(p):
            ht = hpool.tile([P, NHID // P, PAIR], BF16)
            hts[p] = ht
            hps = {}
            for k in range(NHID // P):
                for j in range(2):
                    col = p * PAIR + j * CHUNK
                    hp = hpsum.tile([P, CHUNK], F32, tag=f"h{k}{j}", name=f"hp{k}{j}")
                    hps[k, j] = hp
                    nc.tensor.matmul(
                        out=hp[:],
                        lhsT=w1t_t[:, k * P:(k + 1) * P],
                        rhs=xt[:, 0, col:col + CHUNK],
                        start=True,
                        stop=True,
                    )
            for k in range(NHID // P):
                for j in range(2):
                    dst = ht[:, k, j * CHUNK:(j + 1) * CHUNK]
                    if k == 0:
                        nc.scalar.activation(
                            out=dst, in_=hps[k, j][:],
                            func=mybir.ActivationFunctionType.Relu,
                            bias=b1_t[:, k:k + 1],
                        )
                    else:
                        nc.vector.tensor_scalar(
                            out=dst, in0=hps[k, j][:],
                            scalar1=b1_t[:, k:k + 1], scalar2=0.0,
                            op0=mybir.AluOpType.add, op1=mybir.AluOpType.max,
                        )

        def lin2(p):
            ht = hts[p]
            ops = {}
            for k in range(NHID // P):
                for j in range(2):
                    if k == 0:
                        ops[j] = opsum.tile([NOUT, CHUNK], F32, tag=f"ot{j}", name=f"op{j}")
                    nc.tensor.matmul(
                        out=ops[j][:],
                        lhsT=w2t_t[:, k, :],
                        rhs=ht[:, k, j * CHUNK:(j + 1) * CHUNK],
                        start=(k == 0),
                        stop=(k == NHID // P - 1),
                    )
            ot = opool.tile([NOUT, PAIR], BF16)
            for j in range(2):
                dst = ot[:, j * CHUNK:(j + 1) * CHUNK]
                if j == 0:
                    nc.vector.tensor_scalar(
                        out=dst, in0=ops[j][:],
                        scalar1=b2_t[:], scalar2=0.0,
                        op0=mybir.AluOpType.add, op1=mybir.AluOpType.max,
                    )
                else:
                    nc.scalar.activation(
                        out=dst, in_=ops[j][:],
                        func=mybir.ActivationFunctionType.Relu,
                        bias=b2_t[:],
                    )
            # Alternate the two HWDGE rings so output writes run ~2x faster.
            eng = nc.sync if p % 2 == 0 else nc.scalar
            eng.dma_start(outT[:, p * PAIR:(p + 1) * PAIR], ot[:])

        for p in range(NPAIR):
            lin1(p)
            if p >= 1:
                lin2(p - 1)
        lin2(NPAIR - 1)

    nc.compile()
    return nc


TRACE = False          # set by test harness to capture an NTFF profile
RUN_KWARGS = None      # extra kwargs for run_bass_kernel_spmd (test harness)
LAST = None            # last BassKernelResults (test harness reads exec_time_ns)


def _plan_windows(sidx, call_sizes):
    """Per-core window bases for each gather call; None if a window overflows."""
    call_offs = np.concatenate([[0], np.cumsum(call_sizes)]).astype(int)
    bases = np.empty((N_CORES, len(call_sizes)), dtype=np.int64)
    for c in range(N_CORES):
        seg = sidx[c * BC:(c + 1) * BC]
        for k, n in enumerate(call_sizes):
            s = int(call_offs[k])
            lo, hi = int(seg[s]), int(seg[s + n - 1])
            if hi - lo >= WIN:
                return None
            bases[c, k] = min(lo, NROWS - WIN)
    return bases


def kernel(entity_embedding, w1, b1, w2, b2, idx0, idx1):
    table = np.asarray(entity_embedding, dtype=np.float32).reshape(NROWS, FEAT)
    table_bf = table.astype(BF16_NP)
    flat_idx = (np.asarray(idx0, dtype=np.int64) * 4
                + np.asarray(idx1, dtype=np.int64))

    order = np.argsort(flat_idx, kind="stable")
    sidx = flat_idx[order]

    call_sizes = CALL_SIZES
    bases = _plan_windows(sidx, call_sizes)
    if bases is None:
        call_sizes = CALL_SIZES_SAFE
        bases = _plan_windows(sidx, call_sizes)
        assert bases is not None, "index windows overflow even at 1024/call"
    call_offs = np.concatenate([[0], np.cumsum(call_sizes)]).astype(int)

    w1t = np.ascontiguousarray(np.asarray(w1, dtype=np.float32).T).astype(BF16_NP)
    w2t = np.ascontiguousarray(
        np.asarray(w2, dtype=np.float32).T.reshape(NHID // P, P, NOUT)
        .transpose(1, 0, 2)).astype(BF16_NP)
    b1v = np.ascontiguousarray(
        np.asarray(b1, dtype=np.float32).reshape(NHID // P, P).T)
    b2v = np.ascontiguousarray(np.asarray(b2, dtype=np.float32).reshape(NOUT, 1))

    nidxcol = BC // 16
    in_maps = []
    for c in range(N_CORES):
        seg = sidx[c * BC:(c + 1) * BC]
        ltab = np.concatenate(
            [table_bf[bases[c, k]:bases[c, k] + WIN] for k in range(len(call_sizes))],
            axis=0)
        idx16 = np.empty((16, nidxcol), dtype=np.int16)
        for k, n in enumerate(call_sizes):
            s = int(call_offs[k])
            local = (seg[s:s + n] - bases[c, k]).astype(np.int16)
            idx16[:, s // 16:(s + n) // 16] = local.reshape(n // 16, 16).T
        idxs = np.ascontiguousarray(np.tile(idx16, (P // 16, 1)))
        in_maps.append({
            "ltab": ltab,
            "idxs": idxs,
            "w1t": w1t,
            "w2t": w2t,
            "b1v": b1v,
            "b2v": b2v,
        })

    nc = _build_program(call_sizes)
    global LAST
    res = run_bass_kernel_spmd(
        nc, in_maps, core_ids=list(range(N_CORES)), trace=TRACE,
        **(RUN_KWARGS or {}),
    )
    LAST = res
    sorted_out = np.empty((B, NOUT), dtype=np.float32)
    for c in range(N_CORES):
        sorted_out[c * BC:(c + 1) * BC] = (
            np.asarray(res.results[c]["outT"]).astype(np.float32).T)
    out = np.empty((B, NOUT), dtype=np.float32)
    out[order] = sorted_out
    return out


if __name__ == "__main__":
    rng = np.random.default_rng(0)
    ins = {
        "entity_embedding": rng.standard_normal((500000, 4, FEAT), dtype=np.float32),
        "w1": rng.standard_normal((NHID, FEAT), dtype=np.float32) / np.sqrt(FEAT),
        "b1": rng.standard_normal((NHID,), dtype=np.float32) / np.sqrt(FEAT),
        "w2": rng.standard_normal((NOUT, NHID), dtype=np.float32) / np.sqrt(NHID),
        "b2": rng.standard_normal((NOUT,), dtype=np.float32) / np.sqrt(NHID),
        "idx0": rng.integers(0, 500000, B).astype(np.int32),
        "idx1": rng.integers(0, 4, B).astype(np.int32),
    }
    out = kernel(**ins)
    x = ins["entity_embedding"].reshape(NROWS, FEAT)[
        ins["idx0"].astype(np.int64) * 4 + ins["idx1"]]
    h = np.maximum(x @ ins["w1"].T + ins["b1"], 0.0)
    ref = np.maximum(h @ ins["w2"].T + ins["b2"], 0.0)
    err = np.abs(out - ref).max() / max(np.abs(ref).max(), 1e-9)
    print("rel err:", err)
